# revision 21
# baseline (speedup 1.0000x reference)
"""DeepLSTM (B=32, T=512, I=256, H=512, L=4) Trainium2 kernel.

Data-parallel over batch across 8 cores (B_LOC=4 each). bf16 matmul inputs
with fp32 PSUM accumulation. The 4 layers are wavefront-pipelined over
chunks of C=32 timesteps: in round r, layer l processes chunk r-l, so up to
4 independent recurrence chains are in flight and the per-step serial
latency (ACT -> DVE -> transpose -> copy) of one layer hides under the
other layers' matmuls, keeping the PE dense (and HAM-warm).

Per (chunk, layer): the input-side gate contributions for all C steps are
precomputed by one GEMM (stationary = x^T over the 128 (t,b) columns,
moving = Wx, bias folded in via a K=1 matmul). Per step, the
x-contribution is injected into the gate PSUM with an identity-column
matmul, then 16 h-side matmuls (4 k-chunks x 4 strips in parallel column
groups) accumulate h_{t-1} @ Wh.
"""
import sys

if '/opt/trn_rl_repo' not in sys.path:
    sys.path.insert(0, '/opt/trn_rl_repo')

import numpy as np
import ml_dtypes

BF16 = ml_dtypes.bfloat16

B, T, I, H, L = 32, 512, 256, 512, 4
N_CORES = 8
B_LOC = B // N_CORES
C = 32           # timesteps per chunk
NS = 4           # strips (H/128)
G = 4            # gates
KC = 4           # k-chunks of 128
KC0 = 2          # k-chunks for layer-0 input (I=256)
NCH = T // C     # chunks


def _pack_w(wlist_g, K, kc):
    # [G, K, H] -> [kc, 128, NS, G, 128]  (k, p, j, g, u)
    W4 = np.stack(wlist_g, axis=0).astype(np.float32)
    if K < kc * 128:
        W4 = np.concatenate(
            [W4, np.zeros((G, kc * 128 - K, H), np.float32)], axis=1)
    W5 = W4.reshape(G, kc, 128, NS, 128)
    return np.ascontiguousarray(W5.transpose(1, 2, 3, 0, 4))


def _pack_xT(x_shard):
    # [B_LOC, T, I] -> [KC0, 128, T*B_LOC]   (k, p, (t b))
    B_l, T_, I_ = x_shard.shape
    xt = x_shard.reshape(B_l, T_, KC0, 128).transpose(2, 3, 1, 0)
    return np.ascontiguousarray(xt.reshape(KC0, 128, T_ * B_l))


_NC_CACHE = {}


def _build_nc():
    if 'nc' in _NC_CACHE:
        return _NC_CACHE['nc']
    import concourse.bacc as bacc
    import concourse.tile as tile
    import concourse.mybir as mybir
    from concourse.masks import make_identity

    f32 = mybir.dt.float32
    bf16 = mybir.dt.bfloat16
    AF = mybir.ActivationFunctionType

    nc = bacc.Bacc("TRN2", target_bir_lowering=False, debug=False)
    wh_dram = nc.dram_tensor("wh", [L, KC, 128, NS, G, 128], bf16,
                             kind="ExternalInput")
    wx_dram = nc.dram_tensor("wx", [L - 1, KC, 128, NS, G, 128], bf16,
                             kind="ExternalInput")
    wx0_dram = nc.dram_tensor("wx0", [KC0, 128, NS, G, 128], bf16,
                              kind="ExternalInput")
    xt_dram = nc.dram_tensor("xt", [KC0, 128, T * B_LOC], bf16,
                             kind="ExternalInput")
    id_dram = nc.dram_tensor("idm", [128, 128], bf16, kind="ExternalInput")
    bias_dram = nc.dram_tensor("biasp", [1, L, G, NS * 128], bf16,
                               kind="ExternalInput")
    out_dram = nc.dram_tensor("ht_out", [KC, 128, T * B_LOC], bf16,
                              kind="ExternalOutput")

    with tile.TileContext(nc) as tc:
        with tc.tile_pool(name="persist", bufs=1) as pp, \
             tc.tile_pool(name="xtp", bufs=2) as xtp, \
             tc.tile_pool(name="xcp", bufs=1) as xcp, \
             tc.tile_pool(name="work", bufs=2) as wk, \
             tc.tile_pool(name="psg", bufs=1, space="PSUM") as psg, \
             tc.tile_pool(name="psx", bufs=2, space="PSUM") as psx:

        # ---------- persistent state ----------
            identb = pp.tile([128, 128], bf16)
            nc.sync.dma_start(out=identb[:], in_=id_dram[:])
            ident_f = pp.tile([128, 128], f32)
            make_identity(nc, ident_f[:])
            bias_sb = pp.tile([1, L, G, NS * 128], bf16)
            nc.sync.dma_start(out=bias_sb[:], in_=bias_dram[:])
            ones_sb = pp.tile([1, 128], bf16)
            nc.gpsimd.memset(ones_sb[:], 1.0)
            zero_sb = pp.tile([128, G * 128], f32)
            nc.gpsimd.memset(zero_sb[:], 0.0)
            zcol = pp.tile([128, 1], bf16)
            nc.gpsimd.memset(zcol[:], 0.0)

            w_h = [pp.tile([128, KC, NS, G, 128], bf16, name=f"wh{l}")
                   for l in range(L)]
            w_x = [pp.tile([128, KC0 if l == 0 else KC, NS, G, 128], bf16,
                           name=f"wx{l}") for l in range(L)]
            for l in range(L):
                nc.sync.dma_start(
                    out=w_h[l][:],
                    in_=wh_dram[l].rearrange("k p j g u -> p k j g u"))
                if l == 0:
                    nc.sync.dma_start(
                        out=w_x[0][:],
                        in_=wx0_dram.rearrange("k p j g u -> p k j g u"))
                else:
                    nc.sync.dma_start(
                        out=w_x[l][:],
                        in_=wx_dram[l - 1].rearrange("k p j g u -> p k j g u"))

            hist = [[pp.tile([128, KC, C + 1, B_LOC], bf16,
                             name=f"hist{l}_{p}") for p in range(2)]
                    for l in range(L)]
            c_state = [pp.tile([128, 128], f32, name=f"cstate{l}")
                       for l in range(L)]
            for l in range(L):
                nc.gpsimd.memset(hist[l][0][:, :, 0, :], 0.0)
                nc.gpsimd.memset(c_state[l][:], 0.0)

            # zero-init the 4 per-layer gate PSUM banks (ACT reads the
            # 112 partitions the matmuls never write)
            for l in range(L):
                g0 = psg.tile([128, G, 128], f32, tag=f"gates{l}")
                nc.vector.tensor_copy(
                    g0[:, :, :].rearrange("p g u -> p (g u)"), zero_sb[:])

            xt_tiles = {}
            xc_cur = {}
            gates_cur = {}
            gs_cur = {}
            th_cur = {}
            h_cur = {}
            tp_cur = {}

            def emit_xgemm(l, i):
                xc_t = xcp.tile([128, G, NS * 128], bf16, tag=f"xc{l}")
                nk = KC0 if l == 0 else KC
                for g in range(G):
                    xg = psx.tile([128, NS * 128], f32, tag="xg")
                    for k in range(nk):
                        stat = (xt_tiles[i % 2][:, k, :] if l == 0
                                else hist[l - 1][i % 2][:, k, 1:C + 1, :])
                        nc.tensor.matmul(
                            xg[:], stat, w_x[l][:, k, :, g, :],
                            start=(k == 0), stop=False)
                    nc.tensor.matmul(xg[:], ones_sb[:], bias_sb[:, l, g, :],
                                     start=False, stop=True)
                    nc.scalar.copy(xc_t[:, g, :], xg[:])
                xc_cur[l] = xc_t

            def emit_step_mms(l, i, t):
                gates = psg.tile([128, G, 128], f32, tag=f"gates{l}")
                for j in range(NS):
                    nc.tensor.matmul(
                        gates[32 * j:32 * j + B_LOC, :, :],
                        identb[:, 4 * t:4 * t + B_LOC],
                        xc_cur[l][:, :, 128 * j:128 * j + 128],
                        start=True, stop=False,
                        tile_position=(0, 32 * j))
                for k in range(KC):
                    for j in range(NS):
                        nc.tensor.matmul(
                            gates[32 * j:32 * j + B_LOC, :, :],
                            hist[l][i % 2][:, k, t, :],
                            w_h[l][:, k, j, :, :],
                            start=False, stop=(k == KC - 1),
                            tile_position=(0, 32 * j))
                gates_cur[l] = gates

            def emit_act1(l):
                gs = wk.tile([128, G, 128], bf16, tag=f"gs{l}")
                nc.scalar.activation(gs[:, 0:3, :], gates_cur[l][:, 0:3, :],
                                     AF.Sigmoid)
                nc.scalar.activation(gs[:, 3, :], gates_cur[l][:, 3, :],
                                     AF.Tanh)
                gs_cur[l] = gs

            def emit_cmul(l):
                gs = gs_cur[l]
                fc = wk.tile([128, 128], f32, tag=f"fc{l}")
                ic = wk.tile([128, 128], f32, tag=f"ic{l}")
                nc.vector.tensor_mul(fc[:], gs[:, 1, :], c_state[l][:])
                nc.vector.tensor_mul(ic[:], gs[:, 0, :], gs[:, 3, :])
                nc.vector.tensor_add(c_state[l][:], fc[:], ic[:])

            def emit_act2(l):
                th = wk.tile([128, 128], bf16, tag=f"th{l}")
                nc.scalar.activation(th[:], c_state[l][:], AF.Tanh)
                th_cur[l] = th

            def emit_hmul(l):
                h_sb = wk.tile([128, 128], bf16, tag=f"h{l}")
                nc.vector.tensor_mul(h_sb[:], gs_cur[l][:, 2, :], th_cur[l][:])
                h_cur[l] = h_sb

            def emit_tr(l):
                tp = psx.tile([128, NS, 32], bf16, tag="tp")
                nc.tensor.transpose(
                    tp[:, :, :].rearrange("p j b -> p (j b)"),
                    h_cur[l][:], identb[:])
                tp_cur[l] = tp

            def emit_cast(l, i, t):
                eng = nc.scalar if l % 2 == 0 else nc.vector
                if l % 2 == 0:
                    nc.scalar.copy(hist[l][i % 2][:, :, t + 1, :],
                                   tp_cur[l][:, :, 0:B_LOC])
                else:
                    nc.vector.tensor_copy(hist[l][i % 2][:, :, t + 1, :],
                                          tp_cur[l][:, :, 0:B_LOC])

            # ---------- wavefront over rounds ----------
            # prefetch chunk 0
            xt_tiles[0] = xtp.tile([128, KC0, C * B_LOC], bf16, tag="xt",
                                   name="xtb")
            nc.sync.dma_start(
                out=xt_tiles[0][:],
                in_=xt_dram.rearrange("k p n -> p k n")[:, :, 0:C * B_LOC])

            for r in range(NCH + L - 1):
                if r + 1 < NCH:
                    ch = r + 1
                    xt_tiles[ch % 2] = xtp.tile([128, KC0, C * B_LOC], bf16,
                                                tag="xt", name="xtb")
                    nc.sync.dma_start(
                        out=xt_tiles[ch % 2][:],
                        in_=xt_dram.rearrange("k p n -> p k n")
                            [:, :, ch * C * B_LOC:(ch + 1) * C * B_LOC])
                pairs = [(l, r - l) for l in range(L) if 0 <= r - l < NCH]
                for (l, i) in pairs:
                    emit_xgemm(l, i)
                for t in range(C):
                    for (l, i) in pairs:
                        emit_step_mms(l, i, t)
                    for (l, i) in pairs:
                        emit_act1(l)
                        emit_cmul(l)
                        emit_act2(l)
                        emit_hmul(l)
                    for (l, i) in pairs:
                        emit_tr(l)
                    for (l, i) in pairs:
                        emit_cast(l, i, t)
                for (l, i) in pairs:
                    nc.vector.tensor_copy(hist[l][(i + 1) % 2][:, :, 0, :],
                                          hist[l][i % 2][:, :, C, :])
                    if l == L - 1:
                        nc.sync.dma_start(
                            out=out_dram.rearrange("k p n -> p k n")
                                [:, :, i * C * B_LOC:(i + 1) * C * B_LOC],
                            in_=hist[l][i % 2][:, :, 1:C + 1, :].rearrange(
                                "p k t b -> p k (t b)"),
                        )
    nc.compile()
    _NC_CACHE['nc'] = nc
    return nc


def kernel(inputs, Wxi0, Wxf0, Wxo0, Wxc0, Wxi, Wxf, Wxo, Wxc,
           Whi, Whf, Who, Whc, bi, bf, bo, bc, _trace=False, _tmpdir=None):
    from concourse.bass_utils import run_bass_kernel_spmd

    inputs = np.asarray(inputs, dtype=np.float32)
    Wx_l = [[np.asarray(Wxi0), np.asarray(Wxf0), np.asarray(Wxo0),
             np.asarray(Wxc0)]]
    for li in range(L - 1):
        Wx_l.append([np.asarray(Wxi)[li], np.asarray(Wxf)[li],
                     np.asarray(Wxo)[li], np.asarray(Wxc)[li]])
    Wh_l = [[np.asarray(Whi)[li], np.asarray(Whf)[li], np.asarray(Who)[li],
             np.asarray(Whc)[li]] for li in range(L)]
    b_l = [[np.asarray(bi)[li], np.asarray(bf)[li], np.asarray(bo)[li],
            np.asarray(bc)[li]] for li in range(L)]

    wh = np.stack([_pack_w(Wh_l[l], H, KC) for l in range(L)]).astype(BF16)
    wx = np.stack([_pack_w(Wx_l[l], H, KC)
                   for l in range(1, L)]).astype(BF16)
    wx0 = _pack_w(Wx_l[0], I, KC0).astype(BF16)

    biasp = np.zeros((1, L, G, NS * 128), np.float32)
    for l in range(L):
        for g in range(G):
            biasp[0, l, g] = b_l[l][g].astype(np.float32)
    biasp = biasp.astype(BF16)

    idm = np.eye(128, dtype=np.float32).astype(BF16)

    nc = _build_nc()
    in_maps = []
    for cid in range(N_CORES):
        shard = inputs[cid * B_LOC:(cid + 1) * B_LOC]
        in_maps.append({
            "wh": wh,
            "wx": wx,
            "wx0": wx0,
            "xt": _pack_xT(shard).astype(BF16),
            "idm": idm,
            "biasp": biasp,
        })
    res = run_bass_kernel_spmd(nc, in_maps, core_ids=list(range(N_CORES)),
                               trace=_trace, tmpdir=_tmpdir)
    out = np.zeros((B, T, H), np.float32)
    for cid in range(N_CORES):
        ht = np.asarray(res.results[cid]["ht_out"]).astype(
            np.float32).reshape(KC, 128, T, B_LOC)
        out[cid * B_LOC:(cid + 1) * B_LOC] = ht.transpose(3, 2, 0, 1).reshape(
            B_LOC, T, H)
    if _trace:
        _NC_CACHE['last_result'] = res
    return out


# revision 22
# speedup vs baseline: 1.2331x; 1.2331x over previous
"""DeepLSTM (B=32, T=512, I=256, H=512, L=4) Trainium2 kernel.

Data-parallel over batch across 8 cores (B_LOC=4 each). bf16 matmul inputs
with fp32 PSUM accumulation. The 4 layers are wavefront-pipelined over
chunks of C=32 timesteps: in round r, layer l processes chunk r-l, so up to
4 independent recurrence chains are in flight and the per-step serial
latency (ACT -> DVE -> transpose -> copy) of one layer hides under the
other layers' matmuls, keeping the PE dense (and HAM-warm).

Per (chunk, layer): the input-side gate contributions for all C steps are
precomputed by one GEMM (stationary = x^T over the 128 (t,b) columns,
moving = Wx, bias folded in via a K=1 matmul). Per step, the
x-contribution is injected into the gate PSUM with an identity-column
matmul, then 16 h-side matmuls (4 k-chunks x 4 strips in parallel column
groups) accumulate h_{t-1} @ Wh.
"""
import sys

if '/opt/trn_rl_repo' not in sys.path:
    sys.path.insert(0, '/opt/trn_rl_repo')

import numpy as np
import ml_dtypes

BF16 = ml_dtypes.bfloat16

B, T, I, H, L = 32, 512, 256, 512, 4
N_CORES = 8
B_LOC = B // N_CORES
C = 32           # timesteps per chunk
NS = 4           # strips (H/128)
G = 4            # gates
KC = 4           # k-chunks of 128
KC0 = 2          # k-chunks for layer-0 input (I=256)
NCH = T // C     # chunks


def _pack_w(wlist_g, K, kc):
    # [G, K, H] -> [kc, 128, NS, G, 128]  (k, p, j, g, u)
    W4 = np.stack(wlist_g, axis=0).astype(np.float32)
    if K < kc * 128:
        W4 = np.concatenate(
            [W4, np.zeros((G, kc * 128 - K, H), np.float32)], axis=1)
    W5 = W4.reshape(G, kc, 128, NS, 128)
    return np.ascontiguousarray(W5.transpose(1, 2, 3, 0, 4))


def _pack_xT(x_shard):
    # [B_LOC, T, I] -> [KC0, 128, T*B_LOC]   (k, p, (t b))
    B_l, T_, I_ = x_shard.shape
    xt = x_shard.reshape(B_l, T_, KC0, 128).transpose(2, 3, 1, 0)
    return np.ascontiguousarray(xt.reshape(KC0, 128, T_ * B_l))


_NC_CACHE = {}


def _build_nc():
    if 'nc' in _NC_CACHE:
        return _NC_CACHE['nc']
    import concourse.bacc as bacc
    import concourse.tile as tile
    import concourse.mybir as mybir
    from concourse.masks import make_identity

    f32 = mybir.dt.float32
    bf16 = mybir.dt.bfloat16
    AF = mybir.ActivationFunctionType

    nc = bacc.Bacc("TRN2", target_bir_lowering=False, debug=False)
    wh_dram = nc.dram_tensor("wh", [L, KC, 128, NS, G, 128], bf16,
                             kind="ExternalInput")
    wx_dram = nc.dram_tensor("wx", [L - 1, KC, 128, NS, G, 128], bf16,
                             kind="ExternalInput")
    wx0_dram = nc.dram_tensor("wx0", [KC0, 128, NS, G, 128], bf16,
                              kind="ExternalInput")
    xt_dram = nc.dram_tensor("xt", [KC0, 128, T * B_LOC], bf16,
                             kind="ExternalInput")
    id_dram = nc.dram_tensor("idm", [128, 128], bf16, kind="ExternalInput")
    bias_dram = nc.dram_tensor("biasp", [1, L, G, NS * 128], bf16,
                               kind="ExternalInput")
    out_dram = nc.dram_tensor("ht_out", [KC, 128, T * B_LOC], bf16,
                              kind="ExternalOutput")

    with tile.TileContext(nc) as tc:
        with tc.tile_pool(name="persist", bufs=1) as pp, \
             tc.tile_pool(name="xtp", bufs=2) as xtp, \
             tc.tile_pool(name="xcp", bufs=1) as xcp, \
             tc.tile_pool(name="work", bufs=2) as wk, \
             tc.tile_pool(name="psg", bufs=1, space="PSUM") as psg, \
             tc.tile_pool(name="psx", bufs=2, space="PSUM") as psx:

        # ---------- persistent state ----------
            identb = pp.tile([128, 128], bf16)
            nc.sync.dma_start(out=identb[:], in_=id_dram[:])
            ident_f = pp.tile([128, 128], f32)
            make_identity(nc, ident_f[:])
            bias_sb = pp.tile([1, L, G, NS * 128], bf16)
            nc.sync.dma_start(out=bias_sb[:], in_=bias_dram[:])
            ones_sb = pp.tile([1, 128], bf16)
            nc.gpsimd.memset(ones_sb[:], 1.0)
            zero_sb = pp.tile([128, G * 128], f32)
            nc.gpsimd.memset(zero_sb[:], 0.0)
            zcol = pp.tile([128, 1], bf16)
            nc.gpsimd.memset(zcol[:], 0.0)

            w_h = [pp.tile([128, KC, NS, G, 128], bf16, name=f"wh{l}")
                   for l in range(L)]
            w_x = [pp.tile([128, KC0 if l == 0 else KC, NS, G, 128], bf16,
                           name=f"wx{l}") for l in range(L)]
            for l in range(L):
                nc.sync.dma_start(
                    out=w_h[l][:],
                    in_=wh_dram[l].rearrange("k p j g u -> p k j g u"))
                if l == 0:
                    nc.sync.dma_start(
                        out=w_x[0][:],
                        in_=wx0_dram.rearrange("k p j g u -> p k j g u"))
                else:
                    nc.sync.dma_start(
                        out=w_x[l][:],
                        in_=wx_dram[l - 1].rearrange("k p j g u -> p k j g u"))

            hist = [[pp.tile([128, KC, C + 1, B_LOC], bf16,
                             name=f"hist{l}_{p}") for p in range(2)]
                    for l in range(L)]
            c_state = [pp.tile([128, 128], f32, name=f"cstate{l}")
                       for l in range(L)]
            for l in range(L):
                nc.gpsimd.memset(hist[l][0][:, :, 0, :], 0.0)
                nc.gpsimd.memset(c_state[l][:], 0.0)

            # zero-init the 4 per-layer gate PSUM banks (ACT reads the
            # 112 partitions the matmuls never write)
            for l in range(L):
                g0 = psg.tile([128, G, 128], f32, tag=f"gates{l}")
                nc.vector.tensor_copy(
                    g0[:, :, :].rearrange("p g u -> p (g u)"), zero_sb[:])

            xt_tiles = {}
            xc_cur = {}
            gates_cur = {}
            gs_cur = {}
            th_cur = {}
            h_cur = {}
            tp_cur = {}

            def emit_xgemm(l, i):
                xc_t = xcp.tile([128, G, NS * 128], bf16, tag=f"xc{l}")
                nk = KC0 if l == 0 else KC
                for g in range(G):
                    xg = psx.tile([128, NS * 128], f32, tag="xg")
                    for k in range(nk):
                        stat = (xt_tiles[i % 2][:, k, :] if l == 0
                                else hist[l - 1][i % 2][:, k, 1:C + 1, :])
                        nc.tensor.matmul(
                            xg[:], stat, w_x[l][:, k, :, g, :],
                            start=(k == 0), stop=False)
                    nc.tensor.matmul(xg[:], ones_sb[:], bias_sb[:, l, g, :],
                                     start=False, stop=True)
                    nc.scalar.copy(xc_t[:, g, :], xg[:])
                xc_cur[l] = xc_t

            def emit_step_mms(l, i, t):
                gates = psg.tile([128, G, 128], f32, tag=f"gates{l}")
                for j in range(NS):
                    nc.tensor.matmul(
                        gates[32 * j:32 * j + B_LOC, :, :],
                        identb[:, 4 * t:4 * t + B_LOC],
                        xc_cur[l][:, :, 128 * j:128 * j + 128],
                        start=True, stop=False,
                        tile_position=(0, 32 * j))
                for k in range(KC):
                    for j in range(NS):
                        nc.tensor.matmul(
                            gates[32 * j:32 * j + B_LOC, :, :],
                            hist[l][i % 2][:, k, t, :],
                            w_h[l][:, k, j, :, :],
                            start=False, stop=(k == KC - 1),
                            tile_position=(0, 32 * j))
                gates_cur[l] = gates

            def emit_act1(l):
                gs = wk.tile([128, G, 128], bf16, tag=f"gs{l}")
                nc.scalar.activation(gs[:, 0:3, :], gates_cur[l][:, 0:3, :],
                                     AF.Sigmoid)
                nc.scalar.activation(gs[:, 3, :], gates_cur[l][:, 3, :],
                                     AF.Tanh)
                gs_cur[l] = gs

            def emit_cmul(l):
                gs = gs_cur[l]
                fc = wk.tile([128, 128], f32, tag=f"fc{l}")
                ic = wk.tile([128, 128], f32, tag=f"ic{l}")
                nc.vector.tensor_mul(fc[:], gs[:, 1, :], c_state[l][:])
                nc.vector.tensor_mul(ic[:], gs[:, 0, :], gs[:, 3, :])
                nc.vector.tensor_add(c_state[l][:], fc[:], ic[:])

            def emit_act2(l):
                th = wk.tile([128, 128], bf16, tag=f"th{l}")
                nc.scalar.activation(th[:], c_state[l][:], AF.Tanh)
                th_cur[l] = th

            def emit_hmul(l):
                h_sb = wk.tile([128, 128], bf16, tag=f"h{l}")
                nc.vector.tensor_mul(h_sb[:], gs_cur[l][:, 2, :], th_cur[l][:])
                h_cur[l] = h_sb

            def emit_tr(l):
                tp = psx.tile([128, NS, 32], bf16, tag="tp")
                nc.tensor.transpose(
                    tp[:, :, :].rearrange("p j b -> p (j b)"),
                    h_cur[l][:], identb[:])
                tp_cur[l] = tp

            def emit_cast(l, i, t):
                nc.vector.tensor_copy(hist[l][i % 2][:, :, t + 1, :],
                                      tp_cur[l][:, :, 0:B_LOC])

            # ---------- wavefront over rounds ----------
            # prefetch chunk 0
            xt_tiles[0] = xtp.tile([128, KC0, C * B_LOC], bf16, tag="xt",
                                   name="xtb")
            nc.sync.dma_start(
                out=xt_tiles[0][:],
                in_=xt_dram.rearrange("k p n -> p k n")[:, :, 0:C * B_LOC])

            for r in range(NCH + L - 1):
                if r + 1 < NCH:
                    ch = r + 1
                    xt_tiles[ch % 2] = xtp.tile([128, KC0, C * B_LOC], bf16,
                                                tag="xt", name="xtb")
                    nc.sync.dma_start(
                        out=xt_tiles[ch % 2][:],
                        in_=xt_dram.rearrange("k p n -> p k n")
                            [:, :, ch * C * B_LOC:(ch + 1) * C * B_LOC])
                pairs = [(l, r - l) for l in range(L) if 0 <= r - l < NCH]
                for (l, i) in pairs:
                    emit_xgemm(l, i)
                for t in range(C):
                    for (l, i) in pairs:
                        if t > 0:
                            emit_tr(l)
                            emit_cast(l, i, t - 1)
                        emit_step_mms(l, i, t)
                    for (l, i) in pairs:
                        emit_act1(l)
                        emit_cmul(l)
                        emit_act2(l)
                        emit_hmul(l)
                for (l, i) in pairs:
                    emit_tr(l)
                    emit_cast(l, i, C - 1)
                for (l, i) in pairs:
                    nc.vector.tensor_copy(hist[l][(i + 1) % 2][:, :, 0, :],
                                          hist[l][i % 2][:, :, C, :])
                    if l == L - 1:
                        nc.sync.dma_start(
                            out=out_dram.rearrange("k p n -> p k n")
                                [:, :, i * C * B_LOC:(i + 1) * C * B_LOC],
                            in_=hist[l][i % 2][:, :, 1:C + 1, :].rearrange(
                                "p k t b -> p k (t b)"),
                        )
    nc.compile()
    _NC_CACHE['nc'] = nc
    return nc


def kernel(inputs, Wxi0, Wxf0, Wxo0, Wxc0, Wxi, Wxf, Wxo, Wxc,
           Whi, Whf, Who, Whc, bi, bf, bo, bc, _trace=False, _tmpdir=None):
    from concourse.bass_utils import run_bass_kernel_spmd

    inputs = np.asarray(inputs, dtype=np.float32)
    Wx_l = [[np.asarray(Wxi0), np.asarray(Wxf0), np.asarray(Wxo0),
             np.asarray(Wxc0)]]
    for li in range(L - 1):
        Wx_l.append([np.asarray(Wxi)[li], np.asarray(Wxf)[li],
                     np.asarray(Wxo)[li], np.asarray(Wxc)[li]])
    Wh_l = [[np.asarray(Whi)[li], np.asarray(Whf)[li], np.asarray(Who)[li],
             np.asarray(Whc)[li]] for li in range(L)]
    b_l = [[np.asarray(bi)[li], np.asarray(bf)[li], np.asarray(bo)[li],
            np.asarray(bc)[li]] for li in range(L)]

    wh = np.stack([_pack_w(Wh_l[l], H, KC) for l in range(L)]).astype(BF16)
    wx = np.stack([_pack_w(Wx_l[l], H, KC)
                   for l in range(1, L)]).astype(BF16)
    wx0 = _pack_w(Wx_l[0], I, KC0).astype(BF16)

    biasp = np.zeros((1, L, G, NS * 128), np.float32)
    for l in range(L):
        for g in range(G):
            biasp[0, l, g] = b_l[l][g].astype(np.float32)
    biasp = biasp.astype(BF16)

    idm = np.eye(128, dtype=np.float32).astype(BF16)

    nc = _build_nc()
    in_maps = []
    for cid in range(N_CORES):
        shard = inputs[cid * B_LOC:(cid + 1) * B_LOC]
        in_maps.append({
            "wh": wh,
            "wx": wx,
            "wx0": wx0,
            "xt": _pack_xT(shard).astype(BF16),
            "idm": idm,
            "biasp": biasp,
        })
    res = run_bass_kernel_spmd(nc, in_maps, core_ids=list(range(N_CORES)),
                               trace=_trace, tmpdir=_tmpdir)
    out = np.zeros((B, T, H), np.float32)
    for cid in range(N_CORES):
        ht = np.asarray(res.results[cid]["ht_out"]).astype(
            np.float32).reshape(KC, 128, T, B_LOC)
        out[cid * B_LOC:(cid + 1) * B_LOC] = ht.transpose(3, 2, 0, 1).reshape(
            B_LOC, T, H)
    if _trace:
        _NC_CACHE['last_result'] = res
    return out
